# revision 17
# baseline (speedup 1.0000x reference)
import os
from contextlib import ExitStack

import numpy as np

import concourse.bass as bass
import concourse.mybir as mybir
from concourse.bass_utils import run_bass_kernel_spmd

F32 = mybir.dt.float32
AF = mybir.ActivationFunctionType
OP = mybir.AluOpType

T = 4096
ROWS = 128
NCORES = 8
SIGMAS = (2.5, 4.0, 6.0, 9.0, 14.0)
RMAX = 56
XPW = T + 2 * RMAX
WIN = 16
TC = 1024
NCH = T // TC

LAST_EXEC_NS = None


def _gk(sigma):
    R = max(1, int(4.0 * sigma + 0.5))
    R = min(R, max(1, (T - 1) // 2))
    xs = np.arange(-R, R + 1, dtype=np.float32)
    k = np.exp(np.float32(-0.5) * (xs / np.float32(sigma)) ** 2).astype(np.float32)
    k = k / (k.sum() + np.float32(1e-12))
    return R, [float(v) for v in k]


class Ser:
    """Serial cross-engine scheduler: buffers (engine, emit_fn) in program
    order, then replays per-engine with standalone wait_ge for cross-engine
    deps and then_inc on the last op before each engine switch."""

    def __init__(self):
        self.ops = []  # [eng, fn, is_dma]

    def add(self, eng, fn, dma=False):
        self.ops.append([eng, fn, dma])

    def emit(self, nc, sems):
        # pass 1: decide incs (on last op before engine switch) and waits
        n = len(self.ops)
        incs = [None] * n     # (sem_name, amt)
        waits = [[] for _ in range(n)]  # list of (sem_name, value)
        cnt = {e: 0 for e in sems}
        observed = {e: {o: 0 for o in sems} for e in sems}
        for i, (eng, fn, dma) in enumerate(self.ops):
            prev = self.ops[i - 1] if i > 0 else None
            if prev is not None and prev[0] != eng and incs[i - 1] is None:
                # close out previous engine: inc its sem on its last op
                incs[i - 1] = (prev[0], 1)
                cnt[prev[0]] += 1
            # waits for this op: observe all other engines' current counts
            for o in sems:
                if o != eng and observed[eng][o] < cnt[o]:
                    waits[i].append((o, cnt[o]))
                    observed[eng][o] = cnt[o]
            if dma:
                # every DMA must carry sync info
                incs[i] = (eng, 16)
                cnt[eng] += 16
        if incs[-1] is None:
            last_eng, _, last_dma = self.ops[-1]
            incs[-1] = (last_eng, 1)
            cnt[last_eng] += 1

        # pass 2: replay per engine
        per = {e: [] for e in sems}
        for i, (eng, fn, dma) in enumerate(self.ops):
            per[eng].append((i, fn, waits[i], incs[i]))
        return per, cnt


def _build(W1, b1, W2, b2):
    nc = bass.Bass()
    xpad_d = nc.dram_tensor("xpad", [ROWS, XPW], F32, kind="ExternalInput")
    out_d = nc.dram_tensor("out", [ROWS, T], F32, kind="ExternalOutput")

    w2p = (np.asarray(W2, np.float64) / 0.7)
    b2p = (np.asarray(b2, np.float64) / 0.7)
    W1 = np.asarray(W1, np.float64)
    b1 = np.asarray(b1, np.float64)

    # const APs for activation biases (mirrors Bass.__init__ preamble)
    def reg_const(val):
        key = (F32, val)
        if key not in nc.const_aps.aps:
            t = nc.alloc_sbuf_tensor(
                f"const-f32-u{len(nc.const_aps.aps)}", [128, 1], F32)
            nc.gpsimd.memset(t.ap(), val)
            nc.const_aps.aps[key] = t.ap()

    for j in range(32):
        reg_const(float(b1[j]))
    nc.all_engine_barrier()

    with ExitStack() as ctx:
        XP1 = T + WIN - 1
        sb = lambda name, w: ctx.enter_context(  # noqa: E731
            nc.sbuf_tensor(name, [ROWS, w], F32))

        xpad = sb("xpad_sb", XPW)
        z = sb("z", T)
        lv = sb("lv", T)
        ls = [sb(f"l{s}", TC) for s in range(5)]
        h0 = sb("h0", TC)
        h1 = sb("h1", TC)
        mx = sb("mx", TC)
        tmp = sb("tmp", TC)
        sctx = ExitStack()
        ssb = lambda name, w: sctx.enter_context(  # noqa: E731
            nc.sbuf_tensor(name, [ROWS, w], F32))
        xps = ssb("xps", XP1)
        xps2 = ssb("xps2", XP1)
        A = ssb("A", XP1)
        B = ssb("B", XP1)
        inv15 = ssb("inv15", WIN - 1)

        xv = xpad[:, RMAX:RMAX + T]

        S = Ser()
        V, A_, G = "v", "a", "g"

        # ---- input DMA ----
        S.add(G, lambda g: g.dma_start(xpad[:], xpad_d[:]), dma=True)

        # ---- stats ----
        for t in range(WIN - 1):
            val = float(1.0 / (t + 1 + 1e-12))
            S.add(V, lambda v, t=t, val=val: v.memset(inv15[:, t:t + 1], val))
        S.add(V, lambda v: v.tensor_copy(xps[:, WIN - 1:], xv))
        S.add(V, lambda v: v.tensor_copy(
            xps[:, 0:WIN - 1], xv[:, 0:1].to_broadcast((ROWS, WIN - 1))))
        S.add(A_, lambda a: a.activation(xps2[:], xps[:], AF.Square))

        def win16(src, dst):
            S.add(V, lambda v: v.tensor_add(A[:, 0:4110], src[:, 0:4110], src[:, 1:4111]))
            S.add(V, lambda v: v.tensor_add(B[:, 0:4108], A[:, 0:4108], A[:, 2:4110]))
            S.add(V, lambda v: v.tensor_add(A[:, 0:4104], B[:, 0:4104], B[:, 4:4108]))
            S.add(V, lambda v: v.tensor_add(dst, A[:, 0:T], A[:, 8:8 + T]))

        win16(xps, z[:])     # Sx  -> z
        win16(xps2, lv[:])   # Sx2 -> lv

        mean = A[:, 0:T]
        mean2 = B[:, 0:T]
        s16 = float(1.0 / (16.0 + 1e-12))
        S.add(V, lambda v: v.tensor_scalar_mul(mean, z[:], s16))
        S.add(V, lambda v: v.tensor_scalar_mul(mean2, lv[:], s16))
        S.add(V, lambda v: v.tensor_mul(mean[:, 0:WIN - 1], z[:, 0:WIN - 1], inv15[:]))
        S.add(V, lambda v: v.tensor_mul(mean2[:, 0:WIN - 1], lv[:, 0:WIN - 1], inv15[:]))

        msq = xps[:, 0:T]
        var = xps2[:, 0:T]
        S.add(V, lambda v: v.tensor_mul(msq, mean, mean))
        S.add(V, lambda v: v.tensor_sub(var, mean2, msq))
        S.add(V, lambda v: v.tensor_scalar_max(var, var, 0.0))
        S.add(V, lambda v: v.tensor_scalar_add(var, var, 1e-6))

        sd = xps[:, 0:T]
        rsd = lv[:]
        S.add(A_, lambda a: a.activation(sd, var, AF.Sqrt, bias=0.0))
        S.add(V, lambda v: v.reciprocal(rsd, sd))
        S.add(V, lambda v: v.tensor_sub(z[:], xv, mean))
        S.add(V, lambda v: v.tensor_mul(z[:], z[:], rsd))
        S.add(A_, lambda a: a.activation(lv[:], var, AF.Ln, bias=0.0))

        # ---- gaussian convs on DVE ----
        sctx.close()  # free stats scratch; convs run after stats (serial)
        Ys = [sb(f"Y{s}", T) for s in range(5)]
        for si, sig in enumerate(SIGMAS):
            R, k = _gk(sig)
            base = RMAX - R
            S.add(V, lambda v, si=si, base=base, k0=k[0]:
                  v.tensor_scalar_mul(Ys[si][:], xpad[:, base:base + T], k0))
            for j in range(1, 2 * R + 1):
                S.add(V, lambda v, si=si, o=base + j, kj=k[j]:
                      v.scalar_tensor_tensor(Ys[si][:], xpad[:, o:o + T], kj,
                                             Ys[si][:], OP.mult, OP.add))

        # ---- gating MLP + softmax + mix ----
        nch_run = 1 if os.environ.get("DBG_STAGE") == "2" else NCH
        for cidx in range(nch_run):
            c0 = cidx * TC
            zc = z[:, c0:c0 + TC]
            lvc = lv[:, c0:c0 + TC]
            for s in range(5):
                S.add(V, lambda v, s=s: v.memset(ls[s][:], float(b2p[s])))
            for j in range(32):
                a = float(W1[j, 0]); b = float(W1[j, 1]); cj = float(b1[j])
                h = (h0 if j % 2 == 0 else h1)
                if a == 0.0 and b == 0.0:
                    S.add(A_, lambda e, h=h, cj=cj, zc=zc:
                          e.activation(h[:], zc, AF.Gelu, bias=cj, scale=0.0))
                elif abs(a) >= abs(b):
                    S.add(V, lambda v, h=h, r=b / a, zc=zc, lvc=lvc:
                          v.scalar_tensor_tensor(h[:], lvc, r, zc, OP.mult, OP.add))
                    S.add(A_, lambda e, h=h, cj=cj, sc=a:
                          e.activation(h[:], h[:], AF.Gelu, bias=cj, scale=sc))
                else:
                    S.add(V, lambda v, h=h, r=a / b, zc=zc, lvc=lvc:
                          v.scalar_tensor_tensor(h[:], zc, r, lvc, OP.mult, OP.add))
                    S.add(A_, lambda e, h=h, cj=cj, sc=b:
                          e.activation(h[:], h[:], AF.Gelu, bias=cj, scale=sc))
                for s in range(5):
                    S.add(V, lambda v, s=s, h=h, w=float(w2p[s, j]):
                          v.scalar_tensor_tensor(ls[s][:], h[:], w, ls[s][:],
                                                 OP.mult, OP.add))

            S.add(V, lambda v: v.tensor_tensor(mx[:], ls[0][:], ls[1][:], OP.max))
            for s in (2, 3, 4):
                S.add(V, lambda v, s=s: v.tensor_tensor(mx[:], mx[:], ls[s][:], OP.max))
            for s in range(5):
                S.add(V, lambda v, s=s: v.tensor_sub(ls[s][:], ls[s][:], mx[:]))
            for s in range(5):
                S.add(A_, lambda a, s=s: a.activation(ls[s][:], ls[s][:], AF.Exp))
            S.add(V, lambda v: v.tensor_add(mx[:], ls[0][:], ls[1][:]))
            for s in (2, 3, 4):
                S.add(V, lambda v, s=s: v.tensor_add(mx[:], mx[:], ls[s][:]))
            S.add(V, lambda v: v.reciprocal(mx[:], mx[:]))

            S.add(V, lambda v, c0=c0: v.tensor_mul(h0[:], ls[0][:], Ys[0][:, c0:c0 + TC]))
            for s in range(1, 5):
                S.add(V, lambda v, s=s, c0=c0:
                      v.tensor_mul(tmp[:], ls[s][:], Ys[s][:, c0:c0 + TC]))
                S.add(V, lambda v: v.tensor_add(h0[:], h0[:], tmp[:]))
            S.add(V, lambda v: v.tensor_mul(h0[:], h0[:], mx[:]))
            S.add(G, lambda g, c0=c0: g.dma_start(out_d[:, c0:c0 + TC], h0[:]),
                  dma=True)

        # ---- debug probe: dump intermediates into output quarters ----
        if os.environ.get("DBG_STAGE") == "1":
            Q = T // 4
            S.add(G, lambda g: g.dma_start(out_d[:, 0:Q], z[:, 0:Q]), dma=True)
            S.add(G, lambda g: g.dma_start(out_d[:, Q:2*Q], lv[:, Q:2*Q]), dma=True)
            S.add(G, lambda g: g.dma_start(out_d[:, 2*Q:3*Q], Ys[0][:, 2*Q:3*Q]), dma=True)
            S.add(G, lambda g: g.dma_start(out_d[:, 3*Q:4*Q], Ys[4][:, 3*Q:4*Q]), dma=True)
        elif os.environ.get("DBG_STAGE") == "2":
            S.add(G, lambda g: g.dma_start(out_d[:, 0:TC], ls[0][:]), dma=True)
            S.add(G, lambda g: g.dma_start(out_d[:, TC:2*TC], ls[1][:]), dma=True)
            S.add(G, lambda g: g.dma_start(out_d[:, 2*TC:3*TC], ls[2][:]), dma=True)
            S.add(G, lambda g: g.dma_start(out_d[:, 3*TC:4*TC], mx[:]), dma=True)

        # ---- emit with semaphores ----
        with nc.semaphore("v_sem") as v_sem, \
             nc.semaphore("a_sem") as a_sem, \
             nc.semaphore("g_sem") as g_sem, \
             nc.Block() as block:
            semmap = {"v": v_sem, "a": a_sem, "g": g_sem}
            per, cnt = S.emit(nc, semmap)

            def replay(eng_obj, eng_name):
                for i, fn, ws, inc in per[eng_name]:
                    for (o, val) in ws:
                        eng_obj.wait_ge(semmap[o], val)
                    inst = fn(eng_obj)
                    if inc is not None:
                        inst.then_inc(semmap[inc[0]], inc[1])

            @block.vector
            def _(vector):
                replay(nc.vector, "v")

            @block.scalar
            def _(scalar):
                replay(nc.scalar, "a")

            @block.gpsimd
            def _(gpsimd):
                replay(nc.gpsimd, "g")

    return nc


def kernel(x, W1, b1, W2, b2):
    global LAST_EXEC_NS
    x = np.asarray(x, np.float32)
    B, T_, C = x.shape
    xr = np.ascontiguousarray(np.transpose(x, (0, 2, 1))).reshape(B * C, T_)
    xp = np.pad(xr, ((0, 0), (RMAX, RMAX)), mode="reflect").astype(np.float32)

    nc = _build(W1, b1, W2, b2)
    in_maps = [
        {"xpad": np.ascontiguousarray(xp[i * ROWS:(i + 1) * ROWS])}
        for i in range(NCORES)
    ]
    trace = bool(os.environ.get("KBENCH_TRACE"))
    res = run_bass_kernel_spmd(nc, in_maps, core_ids=list(range(NCORES)),
                               trace=trace)
    LAST_EXEC_NS = getattr(res, "exec_time_ns", None)
    outs = np.concatenate([np.asarray(res.results[i]["out"])
                           for i in range(NCORES)], axis=0)
    return np.ascontiguousarray(
        outs.reshape(B, C, T_).transpose(0, 2, 1)).astype(np.float32)
